# revision 37
# baseline (speedup 1.0000x reference)
"""2-layer GAT (GATConv x2 + log_softmax) on 8 Trainium2 NeuronCores.

Strategy (SPMD across 8 cores — identical program, per-core input data):
  - Nodes partitioned across cores by dst (2500/core); edges routed to their
    dst-owner core, grouped into 20 windows of 128 dst rows; within a window,
    edges fill K*128 slots (slot j -> partition j%128, chunk j//128).
  - Launch A: h = x@W1 (fp32 matmuls) written as a bf16 gather table
    [N, 512]; per-node attention terms asrc/adst = x @ (W1@blockdiag(att))
    written separately (small). Host concatenates shards to the full table and
    expands per-edge alpha = asrc[src]+adst[dst] into a per-slot array.
  - Launch B (layer-1 edge phase): per window, ONE dma_gather pulls all K*128
    h-rows (bf16, 1024B rows, trailing pad slots use negative indices and are
    skipped); DVE builds the one-hot slot->dst selector (compare dstloc with
    iota) and the ex-weighted messages; K scatter matmuls accumulate messages
    and softmax denominators into PSUM; flush: divide, (+b1), ELU,
    @[W2|att2] producing the bf16 layer-2 table [N, 256] plus per-node
    asrc2/adst2 (small).
  - Launch C (layer-2 edge phase): same, with exp(alpha2) folded into the
    selector (H=1) and the denominator via a ones-column matmul; flush:
    divide, (+b2), log_softmax.
  Scatter matmuls run in bf16 (exact one-hot selectors); feature tables are
  bf16; accumulation is fp32 PSUM.
"""
import numpy as np
import ml_dtypes
from contextlib import ExitStack

import concourse.bass as bass
import concourse.tile as tile
from concourse import mybir
from concourse.bass_utils import run_bass_kernel_spmd
from concourse.library_config import mlp as _mlp_lib
from concourse.library_overlay import lower_extended_insts as _lower_ext

F32 = mybir.dt.float32
F32R = mybir.dt.float32r
BF16 = mybir.dt.bfloat16
I32 = mybir.dt.int32
I16 = mybir.dt.int16
AF = mybir.ActivationFunctionType
OP = mybir.AluOpType
P = 128
NCORES = 8
NEG_SLOPE = 0.2
BF = ml_dtypes.bfloat16


def _split_excess_waits(nc, max_waits=1):
    """This walrus build rejects instructions with >~2 sync waits; move excess
    waits onto same-engine wait-only instructions placed just before."""
    cnt = 0
    for f in nc.m.functions:
        for bb in f.blocks:
            new_insts = []
            for inst in bb.instructions:
                si = inst.sync_info
                if si is not None and si.on_wait and len(si.on_wait) > max_waits:
                    waits = list(si.on_wait)
                    extra, keep = waits[:-max_waits], waits[-max_waits:]
                    for w in extra:
                        cnt += 1
                        nop = mybir.InstNoOp(name=f"wsplit-{cnt}-{inst.name}", ins=[], outs=[])
                        nop.engine = inst.engine
                        nop.sync_info = mybir.SyncInfo(on_wait=[w], on_update=[])
                        new_insts.append(nop)
                    si.on_wait = keep
                new_insts.append(inst)
            bb.instructions = new_insts
    return cnt


def _preprocess(edge_index, N, npc):
    """Route edges to dst-owner cores, bucket into 128-row dst windows, assign
    slots (slot j of window w -> partition j%128, chunk j//128), pad every
    window to K*128 slots with dummy row-0 gathers (killed by dstloc=255)."""
    src = np.concatenate([edge_index[0], np.arange(N, dtype=np.int64)])
    dst = np.concatenate([edge_index[1], np.arange(N, dtype=np.int64)])
    npc_pad = ((npc + P - 1) // P) * P
    nw = npc_pad // P
    buckets = [[None] * nw for _ in range(NCORES)]
    for c in range(NCORES):
        lo, hi = c * npc, (c + 1) * npc
        sel = (dst >= lo) & (dst < hi)
        s_c, d_c = src[sel], dst[sel] - lo
        w_c = d_c // P
        for w in range(nw):
            m = w_c == w
            buckets[c][w] = (s_c[m].astype(np.int64), (d_c[m] % P).astype(np.int64))
    cnt_w = [max(len(buckets[c][w][0]) for c in range(NCORES)) for w in range(nw)]
    kreal = [max(1, (c + P - 1) // P) for c in cnt_w]
    K = max(kreal)
    S = K * P
    slot_src = np.zeros((NCORES, nw, S), np.int64)     # pad slots gather row 0
    slot_dst = np.full((NCORES, nw, S), -1, np.int64)  # global dst node id
    dstrow = np.full((NCORES, nw, S), 255, np.int64)   # dst row within window
    for c in range(NCORES):
        for w in range(nw):
            s_w, r_w = buckets[c][w]
            n = len(s_w)
            slot_src[c, w, :n] = s_w
            slot_dst[c, w, :n] = c * npc + w * P + r_w
            dstrow[c, w, :n] = r_w
    # idx arrays (int16, wrapped 16-way, replicated across partition groups)
    SW = S // 16
    idx_w = np.zeros((NCORES, P, nw * SW), np.int16)
    for c in range(NCORES):
        for w in range(nw):
            a = slot_src[c, w].astype(np.int16).reshape(SW, 16)
            idx_w[c, :, w * SW:(w + 1) * SW] = np.tile(a.T, (8, 1))
    # dstloc layout, pair-duplicated for the DVE 2x packed compare:
    # [p, (w*K + k)*2 + {0,1}] = dstrow[w, k*128+p]
    dl = dstrow.reshape(NCORES, nw, K, P).transpose(0, 3, 1, 2).reshape(NCORES, P, nw * K)
    dl = np.repeat(dl, 2, axis=2).astype(BF)
    return K, nw, npc_pad, kreal, slot_src, slot_dst, idx_w, dl


def _expand_pairs(slot_src, slot_dst, asrc, adst, nw, K):
    """Per-slot alpha = asrc[src] + adst[dst] -> [P, nw*K*H] bf16 (0 for pads)."""
    H = asrc.shape[1]
    s = slot_src.reshape(-1)
    d = slot_dst.reshape(-1)
    valid = d >= 0
    vals = np.zeros((s.shape[0], H), np.float32)
    vals[valid] = asrc[s[valid]] + adst[d[valid]]
    # [nw, K, 128, H] -> [128, nw, K, H], pair-duplicated along H for the
    # DVE 2x packed multiply
    out = vals.reshape(nw, K, P, H).transpose(2, 0, 1, 3).reshape(P, nw * K * H)
    return np.repeat(out, 2, axis=1).astype(BF)


def _asd_blockdiag(a_src, a_dst):
    H, C = a_src.shape
    out = np.zeros((H * C, 2 * H), np.float32)
    for h in range(H):
        out[h * C:(h + 1) * C, h] = a_src[h]
        out[h * C:(h + 1) * C, H + h] = a_dst[h]
    return out


SUBCH = 6  # gather chunks (x128 idxs) per dma_gather call; 48 desc/engine


def _pair_bcast(ap, rep):
    """From [..., n, 2] pair AP, build [..., n, rep, 2] with the rep dim at
    stride 0 — keeps the innermost read step-1 so DVE picks the 2x mode."""
    lay = list(ap.ap)
    return bass.AP(ap.tensor, ap.offset, lay[:-1] + [[0, rep], lay[-1]])


_QN = [0]


def _emit_window_gather(nc, G, tab, idx_sb, w, kw, SW, regs, elem):
    """Gather one window's kw*128 rows as ceil(kw/SUBCH) packed dma_gather
    calls (all slots valid; pads gather row 0), round-robin over the 4
    SWDGE queues (each runs on its own Q7 core pair)."""
    for s0 in range(0, kw, SUBCH):
        kk = min(SUBCH, kw - s0)
        lo = s0 * P
        nc.gpsimd.dma_gather(
            out_ap=G[:, s0 * elem:(s0 + kk) * elem].rearrange("p (k d) -> p k d", d=elem),
            in_ap=tab[:],
            idxs_ap=idx_sb[:, w * SW + lo // 16: w * SW + (lo + kk * P) // 16],
            num_idxs=kk * P,
            num_idxs_reg=regs[kk],
            elem_size=elem,
            single_packet=True,
            queue_num=_QN[0],
        )
        _QN[0] = (_QN[0] + 1) % 4


def _build_A(D1, H1, npc_pad):
    """h = x@W1 -> bf16 table [npc_pad, D1]; alphas = x@(W1@Asd) -> f32
    [P, nw*2*H1] (node t*128+p at column t*2*H1)."""
    nw = npc_pad // P
    KB = D1 // P
    nc = bass.Bass("TRN2", target_bir_lowering=False, debug=False, num_devices=NCORES)
    xT = nc.dram_tensor("xT", [D1, npc_pad], F32R, kind="ExternalInput")
    W1 = nc.dram_tensor("W1", [D1, D1], F32R, kind="ExternalInput")
    W1T = nc.dram_tensor("W1T", [D1, D1], F32R, kind="ExternalInput")
    Asd = nc.dram_tensor("Asd", [D1, 2 * H1], F32R, kind="ExternalInput")
    h_tab = nc.dram_tensor("h_tab", [npc_pad, D1], BF16, kind="ExternalOutput")
    aa1 = nc.dram_tensor("aa1", [P, nw * 2 * H1], F32, kind="ExternalOutput")
    with tile.TileContext(nc) as tc:
        with ExitStack() as ctx:
            const = ctx.enter_context(tc.tile_pool(name="const", bufs=1))
            work = ctx.enter_context(tc.tile_pool(name="work", bufs=3))
            ps = ctx.enter_context(tc.tile_pool(name="ps", bufs=2, space="PSUM"))
            ps2 = ctx.enter_context(tc.tile_pool(name="ps2", bufs=2, space="PSUM"))
            # per-node-tile layout: xsb[p, (t*KB + b)*P + j] = xT[b*P+p, t*P+j],
            # loaded tile-by-tile so the first matmuls start immediately
            xsb = const.tile([P, KB * npc_pad], F32R)
            for t_i in range(nw):
                nc.sync.dma_start(
                    out=xsb[:, t_i * KB * P:(t_i + 1) * KB * P].rearrange(
                        "p (b n) -> p b n", b=KB),
                    in_=xT[:, t_i * P:(t_i + 1) * P].rearrange(
                        "(b p) n -> p b n", p=P))
            w1_sb, w1t_sb, asd_sb = [], [], []
            for kb in range(KB):
                t = const.tile([P, D1], F32R, tag=f"w1_{kb}")
                nc.sync.dma_start(out=t[:], in_=W1[kb * P:(kb + 1) * P, :])
                w1_sb.append(t)
                t2 = const.tile([P, D1], F32R, tag=f"w1t_{kb}")
                nc.sync.dma_start(out=t2[:], in_=W1T[kb * P:(kb + 1) * P, :])
                w1t_sb.append(t2)
                t3 = const.tile([P, 2 * H1], F32R, tag=f"asd_{kb}")
                nc.sync.dma_start(out=t3[:], in_=Asd[kb * P:(kb + 1) * P, :])
                asd_sb.append(t3)
            wsd_sb = []
            for ib in range(KB):
                pw = ps2.tile([P, 2 * H1], F32, tag="pa")
                for cb in range(KB):
                    nc.tensor.matmul(out=pw[:], lhsT=w1t_sb[cb][:, ib * P:(ib + 1) * P],
                                     rhs=asd_sb[cb][:], start=cb == 0, stop=cb == KB - 1)
                t = const.tile([P, 2 * H1], F32R, tag=f"wsd_{ib}")
                nc.scalar.activation(out=t[:], in_=pw[:], func=AF.Copy)
                wsd_sb.append(t)
            aa_acc = const.tile([P, nw * 2 * H1], F32)
            for t_i in range(nw):
                ph = ps.tile([P, D1], F32, tag="ph")
                pa = ps2.tile([P, 2 * H1], F32, tag="pa")
                for kb in range(KB):
                    xt = xsb[:, (t_i * KB + kb) * P:(t_i * KB + kb + 1) * P]
                    nc.tensor.matmul(out=ph[:], lhsT=xt, rhs=w1_sb[kb][:],
                                     start=kb == 0, stop=kb == KB - 1)
                    nc.tensor.matmul(out=pa[:], lhsT=xt, rhs=wsd_sb[kb][:],
                                     start=kb == 0, stop=kb == KB - 1)
                stage = work.tile([P, D1], BF16, tag="stage")
                nc.scalar.activation(out=stage[:], in_=ph[:], func=AF.Copy)
                nc.sync.dma_start(out=h_tab[t_i * P:(t_i + 1) * P, :], in_=stage[:])
                nc.vector.tensor_copy(
                    out=aa_acc[:, t_i * 2 * H1:(t_i + 1) * 2 * H1], in_=pa[:])
            nc.sync.dma_start(out=aa1[:, :], in_=aa_acc[:])
    _split_excess_waits(nc)
    return nc


def _build_B(N, D1, H1, OUTC, npc_pad, K, kreal, with_b1):
    """Layer-1 edge phase + [W2|att2] transform producing the layer-2 table."""
    nw = npc_pad // P
    C1 = D1 // H1
    S = K * P
    SW = S // 16
    OB = OUTC // P
    KB = D1 // P
    nc = bass.Bass("TRN2", target_bir_lowering=False, debug=False, num_devices=NCORES,
                   num_swdge_queues=4)
    tab = nc.dram_tensor("tab", [N, D1], BF16, kind="ExternalInput")
    idxs = nc.dram_tensor("idxs", [P, nw * SW], I16, kind="ExternalInput")
    dlt = nc.dram_tensor("dlt", [P, nw * K * 2], BF16, kind="ExternalInput")
    aab = nc.dram_tensor("aab", [P, nw * K * H1 * 2], BF16, kind="ExternalInput")
    iotg = nc.dram_tensor("iotg", [P, K * P], BF16, kind="ExternalInput")
    idf = nc.dram_tensor("idf", [P, P], F32, kind="ExternalInput")
    W2 = nc.dram_tensor("W2", [D1, OUTC], F32R, kind="ExternalInput")
    W2T = nc.dram_tensor("W2T", [OUTC, D1], F32R, kind="ExternalInput")
    A2 = nc.dram_tensor("A2", [OUTC, 2], F32R, kind="ExternalInput")
    if with_b1:
        B1 = nc.dram_tensor("B1", [P, D1], F32, kind="ExternalInput")
    tab2 = nc.dram_tensor("tab2", [npc_pad, OUTC], BF16, kind="ExternalOutput")
    aa2 = nc.dram_tensor("aa2", [P, nw * 2], F32, kind="ExternalOutput")
    with tile.TileContext(nc) as tc:
        with ExitStack() as ctx:
            const = ctx.enter_context(tc.tile_pool(name="const", bufs=1))
            gp = ctx.enter_context(tc.tile_pool(name="gp", bufs=3))
            mp = ctx.enter_context(tc.tile_pool(name="mp", bufs=2))
            cp = ctx.enter_context(tc.tile_pool(name="cp", bufs=2))
            sp = ctx.enter_context(tc.tile_pool(name="sp", bufs=2))
            fp = ctx.enter_context(tc.tile_pool(name="fp", bufs=2))
            ps_po = ctx.enter_context(tc.tile_pool(name="ps_po", bufs=2, space="PSUM"))
            ps_pd = ctx.enter_context(tc.tile_pool(name="ps_pd", bufs=2, space="PSUM"))
            ps_h2 = ctx.enter_context(tc.tile_pool(name="ps_h2", bufs=2, space="PSUM"))
            ps_ct = ctx.enter_context(tc.tile_pool(name="ps_ct", bufs=2, space="PSUM"))

            nc.gpsimd.load_library(_mlp_lib)
            idx_sb = const.tile([P, nw * SW], I16)
            nc.sync.dma_start(out=idx_sb[:], in_=idxs[:, :])
            dl_sb = const.tile([P, nw * K * 2], BF16)
            nc.sync.dma_start(out=dl_sb[:], in_=dlt[:, :])
            aab_sb = const.tile([P, nw * K * H1 * 2], BF16)
            nc.sync.dma_start(out=aab_sb[:], in_=aab[:, :])
            if with_b1:
                bb = const.tile([P, D1], F32)
                nc.sync.dma_start(out=bb[:], in_=B1[:, :])
            iotag = const.tile([P, K * P], BF16)
            nc.sync.dma_start(out=iotag[:], in_=iotg[:, :])
            identF = const.tile([P, P], F32)
            nc.sync.dma_start(out=identF[:], in_=idf[:, :])
            w2t_sb, a2_sb = [], []
            for ob in range(OB):
                t = const.tile([P, D1], F32R, tag=f"w2t_{ob}")
                nc.sync.dma_start(out=t[:], in_=W2T[ob * P:(ob + 1) * P, :])
                w2t_sb.append(t)
                t2 = const.tile([P, 2], F32R, tag=f"a2_{ob}")
                nc.sync.dma_start(out=t2[:], in_=A2[ob * P:(ob + 1) * P, :])
                a2_sb.append(t2)
            w2ext_sb = []
            for ib in range(KB):
                pv = ps_h2.tile([P, OUTC + 2], F32, tag="ph2")
                for ob in range(OB):
                    nc.tensor.matmul(out=pv[:, :2], lhsT=w2t_sb[ob][:, ib * P:(ib + 1) * P],
                                     rhs=a2_sb[ob][:], start=ob == 0, stop=ob == OB - 1)
                t = const.tile([P, OUTC + 2], F32R, tag=f"w2e_{ib}")
                nc.sync.dma_start(out=t[:, :OUTC], in_=W2[ib * P:(ib + 1) * P, :])
                nc.scalar.activation(out=t[:, OUTC:OUTC + 2], in_=pv[:, :2], func=AF.Copy)
                w2ext_sb.append(t)

            kks = set()
            for kw in kreal:
                kks.update(min(SUBCH, kw - s0) for s0 in range(0, kw, SUBCH))
            regs = {kk: nc.gpsimd.to_reg(kk * P) for kk in kks}
            aa2_acc = const.tile([P, nw * 2], F32)

            def flush_b(w, po, pd):
                den = fp.tile([P, H1 * 2], F32, tag="den", name="den")
                nc.vector.tensor_scalar(out=den[:], in0=pd[:], scalar1=1e-16,
                                        scalar2=None, op0=OP.add)
                den_r = fp.tile([P, H1 * 2], F32, tag="den_r", name="den_r")
                nc.vector.reciprocal(out=den_r[:], in_=den[:])
                dv = den_r[:]
                den_r8 = bass.AP(dv.tensor, dv.offset,
                                 [list(dv.ap)[0], [2, H1], [0, C1]])
                o1 = fp.tile([P, D1], F32, tag="o1", name="o1")
                nc.vector.tensor_tensor(
                    out=o1[:].rearrange("p (h c) -> p h c", h=H1),
                    in0=po[:].rearrange("p (h c) -> p h c", h=H1),
                    in1=den_r8, op=OP.mult)
                if with_b1:
                    nc.vector.tensor_tensor(out=o1[:], in0=o1[:], in1=bb[:], op=OP.add)
                ee = fp.tile([P, D1], F32, tag="ee", name="ee")
                nc.scalar.activation(out=ee[:], in_=o1[:], func=AF.Exp)
                em = fp.tile([P, D1], F32, tag="em", name="em")
                nc.vector.tensor_scalar(out=em[:], in0=ee[:], scalar1=-1.0,
                                        scalar2=None, op0=OP.add)
                nc.vector.tensor_scalar(out=em[:], in0=em[:], scalar1=0.0,
                                        scalar2=None, op0=OP.min)
                h2 = fp.tile([P, D1], F32, tag="h2", name="h2")
                nc.vector.tensor_tensor(out=h2[:], in0=o1[:], in1=em[:], op=OP.max)
                ph2 = ps_h2.tile([P, OUTC + 2], F32, tag="ph2", name="ph2")
                for cb in range(KB):
                    pt = ps_ct.tile([P, P], F32, tag="ct", name="pt")
                    nc.tensor.transpose(out=pt[:], in_=h2[:, cb * P:(cb + 1) * P],
                                        identity=identF[:])
                    h2t = cp.tile([P, P], F32R, tag="h2t", name="h2t")
                    nc.scalar.activation(out=h2t[:], in_=pt[:], func=AF.Copy)
                    nc.tensor.matmul(out=ph2[:], lhsT=h2t[:], rhs=w2ext_sb[cb][:],
                                     start=cb == 0, stop=cb == KB - 1)
                stage = fp.tile([P, OUTC], BF16, tag="stage", name="stage")
                nc.scalar.activation(out=stage[:], in_=ph2[:, :OUTC], func=AF.Copy)
                nc.sync.dma_start(out=tab2[w * P:(w + 1) * P, :], in_=stage[:])
                nc.vector.tensor_copy(out=aa2_acc[:, w * 2:(w + 1) * 2],
                                      in_=ph2[:, OUTC:OUTC + 2])

            pend = None
            for w in range(nw):
                kw = kreal[w]
                G = gp.tile([P, K * D1], BF16, tag="G")
                _emit_window_gather(nc, G, tab, idx_sb, w, kw, SW, regs, D1)
                lr = sp.tile([P, K * H1 * 2], BF16, tag="lr")
                nc.scalar.activation(out=lr[:, :kw * H1 * 2],
                                     in_=aab_sb[:, w * K * H1 * 2: (w * K + kw) * H1 * 2],
                                     func=AF.Prelu, alpha=NEG_SLOPE)
                ex = sp.tile([P, K * H1 * 2], BF16, tag="ex")
                nc.scalar.activation(out=ex[:, :kw * H1 * 2], in_=lr[:, :kw * H1 * 2],
                                     func=AF.Exp)
                CMP = cp.tile([P, K * P], BF16, tag="CMP")
                nc.vector.tensor_tensor(
                    out=CMP[:, :kw * P].rearrange("p (k q2 two) -> p k q2 two",
                                                  k=kw, q2=P // 2),
                    in0=iotag[:, :kw * P].rearrange("p (k q2 two) -> p k q2 two",
                                                    k=kw, q2=P // 2),
                    in1=_pair_bcast(
                        dl_sb[:, w * K * 2: (w * K + kw) * 2]
                        .rearrange("p (k two) -> p k two", k=kw), P // 2),
                    op=OP.is_equal)
                M = mp.tile([P, K * D1], BF16, tag="M")
                nc.vector.tensor_tensor(
                    out=M[:, :kw * D1].rearrange("p (k h c2 two) -> p k h c2 two",
                                                 k=kw, h=H1, c2=C1 // 2),
                    in0=G[:, :kw * D1].rearrange("p (k h c2 two) -> p k h c2 two",
                                                 k=kw, h=H1, c2=C1 // 2),
                    in1=_pair_bcast(
                        ex[:, :kw * H1 * 2]
                        .rearrange("p (k h two) -> p k h two", k=kw, h=H1), C1 // 2),
                    op=OP.mult)
                po = ps_po.tile([P, D1], F32, tag="po")
                pd = ps_pd.tile([P, H1 * 2], F32, tag="pd")
                for k in range(kw):
                    nc.tensor.matmul(out=po[:], lhsT=CMP[:, k * P:(k + 1) * P],
                                     rhs=M[:, k * D1:(k + 1) * D1],
                                     start=k == 0, stop=k == kw - 1)
                    nc.tensor.matmul(out=pd[:], lhsT=CMP[:, k * P:(k + 1) * P],
                                     rhs=ex[:, k * H1 * 2:(k + 1) * H1 * 2],
                                     start=k == 0, stop=k == kw - 1)
                if pend is not None:
                    flush_b(*pend)
                pend = (w, po, pd)
            flush_b(*pend)
            nc.sync.dma_start(out=aa2[:, :], in_=aa2_acc[:])
    _split_excess_waits(nc)
    _lower_ext(nc)
    return nc


def _build_C(N, OUTC, npc_pad, K, kreal, with_b2):
    """Layer-2 edge phase: exp(alpha2) folded into the selector; denominator
    via a ones column; flush = divide, (+b2), log_softmax."""
    nw = npc_pad // P
    S = K * P
    SW = S // 16
    nc = bass.Bass("TRN2", target_bir_lowering=False, debug=False, num_devices=NCORES,
                   num_swdge_queues=4)
    tab2 = nc.dram_tensor("tab2", [N, OUTC], BF16, kind="ExternalInput")
    idxs = nc.dram_tensor("idxs", [P, nw * SW], I16, kind="ExternalInput")
    dlt = nc.dram_tensor("dlt", [P, nw * K * 2], BF16, kind="ExternalInput")
    aab2 = nc.dram_tensor("aab2", [P, nw * K * 2], BF16, kind="ExternalInput")
    iotg = nc.dram_tensor("iotg", [P, K * P], BF16, kind="ExternalInput")
    if with_b2:
        B2 = nc.dram_tensor("B2", [P, OUTC], F32, kind="ExternalInput")
    out_t = nc.dram_tensor("out", [npc_pad, OUTC], F32, kind="ExternalOutput")
    with tile.TileContext(nc) as tc:
        with ExitStack() as ctx:
            const = ctx.enter_context(tc.tile_pool(name="const", bufs=1))
            gp = ctx.enter_context(tc.tile_pool(name="gp", bufs=3))
            cp = ctx.enter_context(tc.tile_pool(name="cp", bufs=2))
            sp = ctx.enter_context(tc.tile_pool(name="sp", bufs=2))
            fp = ctx.enter_context(tc.tile_pool(name="fp", bufs=2))
            ps_po = ctx.enter_context(tc.tile_pool(name="ps_po", bufs=2, space="PSUM"))
            ps_pd = ctx.enter_context(tc.tile_pool(name="ps_pd", bufs=2, space="PSUM"))

            nc.gpsimd.load_library(_mlp_lib)
            idx_sb = const.tile([P, nw * SW], I16)
            nc.sync.dma_start(out=idx_sb[:], in_=idxs[:, :])
            dl_sb = const.tile([P, nw * K * 2], BF16)
            nc.sync.dma_start(out=dl_sb[:], in_=dlt[:, :])
            aab_sb = const.tile([P, nw * K * 2], BF16)
            nc.sync.dma_start(out=aab_sb[:], in_=aab2[:, :])
            if with_b2:
                bb = const.tile([P, OUTC], F32)
                nc.sync.dma_start(out=bb[:], in_=B2[:, :])
            iotag = const.tile([P, K * P], BF16)
            nc.sync.dma_start(out=iotag[:], in_=iotg[:, :])
            ones = const.tile([P, 1], BF16)
            nc.vector.memset(ones[:], 1.0)

            kks = set()
            for kw in kreal:
                kks.update(min(SUBCH, kw - s0) for s0 in range(0, kw, SUBCH))
            regs = {kk: nc.gpsimd.to_reg(kk * P) for kk in kks}

            def flush_c(w, po, pd):
                den = fp.tile([P, 1], F32, tag="den", name="den")
                nc.vector.tensor_scalar(out=den[:], in0=pd[:], scalar1=1e-16,
                                        scalar2=None, op0=OP.add)
                den_r = fp.tile([P, 1], F32, tag="den_r", name="den_r")
                nc.vector.reciprocal(out=den_r[:], in_=den[:])
                z = fp.tile([P, OUTC], F32, tag="z", name="z")
                nc.vector.tensor_scalar(out=z[:], in0=po[:], scalar1=den_r[:, :1],
                                        scalar2=None, op0=OP.mult)
                if with_b2:
                    nc.vector.tensor_tensor(out=z[:], in0=z[:], in1=bb[:], op=OP.add)
                ee = fp.tile([P, OUTC], F32, tag="ee", name="ee")
                se = fp.tile([P, 1], F32, tag="se", name="se")
                nc.scalar.activation(out=ee[:], in_=z[:], func=AF.Exp, accum_out=se[:])
                lse = fp.tile([P, 1], F32, tag="lse", name="lse")
                nc.scalar.activation(out=lse[:], in_=se[:], func=AF.Ln)
                nc.vector.tensor_scalar(out=z[:], in0=z[:], scalar1=lse[:, :1],
                                        scalar2=None, op0=OP.subtract)
                nc.sync.dma_start(out=out_t[w * P:(w + 1) * P, :], in_=z[:])

            pend = None
            for w in range(nw):
                kw = kreal[w]
                G = gp.tile([P, K * OUTC], BF16, tag="G")
                _emit_window_gather(nc, G, tab2, idx_sb, w, kw, SW, regs, OUTC)
                lr = sp.tile([P, K * 2], BF16, tag="lr")
                nc.scalar.activation(out=lr[:, :kw * 2],
                                     in_=aab_sb[:, w * K * 2: (w * K + kw) * 2],
                                     func=AF.Prelu, alpha=NEG_SLOPE)
                ex = sp.tile([P, K * 2], BF16, tag="ex")
                nc.scalar.activation(out=ex[:, :kw * 2], in_=lr[:, :kw * 2], func=AF.Exp)
                CMP = cp.tile([P, K * P], BF16, tag="CMP")
                nc.vector.tensor_tensor(
                    out=CMP[:, :kw * P].rearrange("p (k q2 two) -> p k q2 two",
                                                  k=kw, q2=P // 2),
                    in0=iotag[:, :kw * P].rearrange("p (k q2 two) -> p k q2 two",
                                                    k=kw, q2=P // 2),
                    in1=_pair_bcast(
                        dl_sb[:, w * K * 2: (w * K + kw) * 2]
                        .rearrange("p (k two) -> p k two", k=kw), P // 2),
                    op=OP.is_equal)
                CMX = cp.tile([P, K * P], BF16, tag="CMX")
                nc.vector.tensor_tensor(
                    out=CMX[:, :kw * P].rearrange("p (k q2 two) -> p k q2 two",
                                                  k=kw, q2=P // 2),
                    in0=CMP[:, :kw * P].rearrange("p (k q2 two) -> p k q2 two",
                                                  k=kw, q2=P // 2),
                    in1=_pair_bcast(
                        ex[:, :kw * 2].rearrange("p (k two) -> p k two", k=kw), P // 2),
                    op=OP.mult)
                po = ps_po.tile([P, OUTC], F32, tag="po")
                pd = ps_pd.tile([P, 1], F32, tag="pd")
                for k in range(kw):
                    nc.tensor.matmul(out=po[:], lhsT=CMX[:, k * P:(k + 1) * P],
                                     rhs=G[:, k * OUTC:(k + 1) * OUTC],
                                     start=k == 0, stop=k == kw - 1)
                    nc.tensor.matmul(out=pd[:], lhsT=CMX[:, k * P:(k + 1) * P],
                                     rhs=ones[:], start=k == 0, stop=k == kw - 1)
                if pend is not None:
                    flush_c(*pend)
                pend = (w, po, pd)
            flush_c(*pend)
    _split_excess_waits(nc)
    _lower_ext(nc)
    return nc


def kernel(x, edge_index, W1, att_src1, att_dst1, b1, W2, att_src2, att_dst2, b2):
    x = np.asarray(x, np.float32)
    edge_index = np.asarray(edge_index)
    W1 = np.asarray(W1, np.float32)
    W2 = np.asarray(W2, np.float32)
    att_src1 = np.asarray(att_src1, np.float32)
    att_dst1 = np.asarray(att_dst1, np.float32)
    att_src2 = np.asarray(att_src2, np.float32)
    att_dst2 = np.asarray(att_dst2, np.float32)
    b1 = np.asarray(b1, np.float32)
    b2 = np.asarray(b2, np.float32)
    N, D1 = x.shape
    H1 = att_src1.shape[0]
    OUTC = W2.shape[1]
    npc = N // NCORES
    core_ids = list(range(NCORES))
    with_b1 = bool(np.any(b1))
    with_b2 = bool(np.any(b2))

    K, nw, npc_pad, kreal, slot_src, slot_dst, idx_w, dl = _preprocess(edge_index, N, npc)
    asd = _asd_blockdiag(att_src1, att_dst1)
    a2 = np.stack([att_src2[0], att_dst2[0]], axis=1)

    # ---- launch A
    nc_a = _build_A(D1, H1, npc_pad)
    in_maps = []
    for c in range(NCORES):
        xo = np.zeros((npc_pad, D1), np.float32)
        xo[:npc] = x[c * npc:(c + 1) * npc]
        in_maps.append({"xT": np.ascontiguousarray(xo.T), "W1": W1,
                        "W1T": np.ascontiguousarray(W1.T), "Asd": asd})
    res_a = run_bass_kernel_spmd(nc_a, in_maps, core_ids)
    h_full = np.concatenate(
        [res_a.results[c]["h_tab"][:npc] for c in range(NCORES)], axis=0)
    # per-node alpha terms [N, H1] each
    asrc1 = np.zeros((N, H1), np.float32)
    adst1 = np.zeros((N, H1), np.float32)
    for c in range(NCORES):
        aa = res_a.results[c]["aa1"].reshape(P, nw, 2 * H1).transpose(1, 0, 2).reshape(npc_pad, 2 * H1)
        asrc1[c * npc:(c + 1) * npc] = aa[:npc, :H1]
        adst1[c * npc:(c + 1) * npc] = aa[:npc, H1:]

    iotg = np.tile(np.arange(P, dtype=np.float32), (P, K)).astype(BF)
    idf = np.eye(P, dtype=np.float32)

    # ---- launch B
    nc_b = _build_B(N, D1, H1, OUTC, npc_pad, K, kreal, with_b1)
    in_maps = []
    for c in range(NCORES):
        m = {"tab": h_full, "idxs": idx_w[c], "dlt": dl[c],
             "aab": _expand_pairs(slot_src[c], slot_dst[c], asrc1, adst1, nw, K),
             "iotg": iotg, "idf": idf,
             "W2": W2, "W2T": np.ascontiguousarray(W2.T), "A2": a2}
        if with_b1:
            m["B1"] = np.tile(b1.reshape(1, D1), (P, 1))
        in_maps.append(m)
    res_b = run_bass_kernel_spmd(nc_b, in_maps, core_ids)
    t2_full = np.concatenate(
        [res_b.results[c]["tab2"][:npc] for c in range(NCORES)], axis=0)
    asrc2 = np.zeros((N, 1), np.float32)
    adst2 = np.zeros((N, 1), np.float32)
    for c in range(NCORES):
        aa = res_b.results[c]["aa2"].reshape(P, nw, 2).transpose(1, 0, 2).reshape(npc_pad, 2)
        asrc2[c * npc:(c + 1) * npc] = aa[:npc, :1]
        adst2[c * npc:(c + 1) * npc] = aa[:npc, 1:]

    # ---- launch C
    nc_c = _build_C(N, OUTC, npc_pad, K, kreal, with_b2)
    in_maps = []
    for c in range(NCORES):
        m = {"tab2": t2_full, "idxs": idx_w[c], "dlt": dl[c],
             "aab2": _expand_pairs(slot_src[c], slot_dst[c], asrc2, adst2, nw, K),
             "iotg": iotg}
        if with_b2:
            m["B2"] = np.tile(b2.reshape(1, OUTC), (P, 1))
        in_maps.append(m)
    res_c = run_bass_kernel_spmd(nc_c, in_maps, core_ids)
    out = np.concatenate(
        [res_c.results[c]["out"][:npc] for c in range(NCORES)], axis=0)
    return out.astype(np.float32)


# revision 38
# speedup vs baseline: 1.0125x; 1.0125x over previous
"""2-layer GAT (GATConv x2 + log_softmax) on 8 Trainium2 NeuronCores.

Strategy (SPMD across 8 cores — identical program, per-core input data):
  - Nodes partitioned across cores by dst (2500/core); edges routed to their
    dst-owner core, grouped into 20 windows of 128 dst rows; within a window,
    edges fill K*128 slots (slot j -> partition j%128, chunk j//128).
  - Launch A: h = x@W1 (fp32 matmuls) written as a bf16 gather table
    [N, 512]; per-node attention terms asrc/adst = x @ (W1@blockdiag(att))
    written separately (small). Host concatenates shards to the full table and
    expands per-edge alpha = asrc[src]+adst[dst] into a per-slot array.
  - Launch B (layer-1 edge phase): per window, ONE dma_gather pulls all K*128
    h-rows (bf16, 1024B rows, trailing pad slots use negative indices and are
    skipped); DVE builds the one-hot slot->dst selector (compare dstloc with
    iota) and the ex-weighted messages; K scatter matmuls accumulate messages
    and softmax denominators into PSUM; flush: divide, (+b1), ELU,
    @[W2|att2] producing the bf16 layer-2 table [N, 256] plus per-node
    asrc2/adst2 (small).
  - Launch C (layer-2 edge phase): same, with exp(alpha2) folded into the
    selector (H=1) and the denominator via a ones-column matmul; flush:
    divide, (+b2), log_softmax.
  Scatter matmuls run in bf16 (exact one-hot selectors); feature tables are
  bf16; accumulation is fp32 PSUM.
"""
import numpy as np
import ml_dtypes
from contextlib import ExitStack

import concourse.bass as bass
import concourse.tile as tile
from concourse import mybir
from concourse.bass_utils import run_bass_kernel_spmd
from concourse.library_config import mlp as _mlp_lib
from concourse.library_overlay import lower_extended_insts as _lower_ext

F32 = mybir.dt.float32
F32R = mybir.dt.float32r
BF16 = mybir.dt.bfloat16
I32 = mybir.dt.int32
I16 = mybir.dt.int16
AF = mybir.ActivationFunctionType
OP = mybir.AluOpType
P = 128
NCORES = 8
NEG_SLOPE = 0.2
BF = ml_dtypes.bfloat16


def _split_excess_waits(nc, max_waits=1):
    """This walrus build rejects instructions with >~2 sync waits; move excess
    waits onto same-engine wait-only instructions placed just before."""
    cnt = 0
    for f in nc.m.functions:
        for bb in f.blocks:
            new_insts = []
            for inst in bb.instructions:
                si = inst.sync_info
                if si is not None and si.on_wait and len(si.on_wait) > max_waits:
                    waits = list(si.on_wait)
                    extra, keep = waits[:-max_waits], waits[-max_waits:]
                    for w in extra:
                        cnt += 1
                        nop = mybir.InstNoOp(name=f"wsplit-{cnt}-{inst.name}", ins=[], outs=[])
                        nop.engine = inst.engine
                        nop.sync_info = mybir.SyncInfo(on_wait=[w], on_update=[])
                        new_insts.append(nop)
                    si.on_wait = keep
                new_insts.append(inst)
            bb.instructions = new_insts
    return cnt


def _preprocess(edge_index, N, npc):
    """Route edges to dst-owner cores, bucket into 128-row dst windows, assign
    slots (slot j of window w -> partition j%128, chunk j//128), pad every
    window to K*128 slots with dummy row-0 gathers (killed by dstloc=255)."""
    src = np.concatenate([edge_index[0], np.arange(N, dtype=np.int64)])
    dst = np.concatenate([edge_index[1], np.arange(N, dtype=np.int64)])
    npc_pad = ((npc + P - 1) // P) * P
    nw = npc_pad // P
    buckets = [[None] * nw for _ in range(NCORES)]
    for c in range(NCORES):
        lo, hi = c * npc, (c + 1) * npc
        sel = (dst >= lo) & (dst < hi)
        s_c, d_c = src[sel], dst[sel] - lo
        w_c = d_c // P
        for w in range(nw):
            m = w_c == w
            buckets[c][w] = (s_c[m].astype(np.int64), (d_c[m] % P).astype(np.int64))
    cnt_w = [max(len(buckets[c][w][0]) for c in range(NCORES)) for w in range(nw)]
    kreal = [max(1, (c + P - 1) // P) for c in cnt_w]
    K = max(kreal)
    S = K * P
    slot_src = np.zeros((NCORES, nw, S), np.int64)     # pad slots gather row 0
    slot_dst = np.full((NCORES, nw, S), -1, np.int64)  # global dst node id
    dstrow = np.full((NCORES, nw, S), 255, np.int64)   # dst row within window
    for c in range(NCORES):
        for w in range(nw):
            s_w, r_w = buckets[c][w]
            n = len(s_w)
            slot_src[c, w, :n] = s_w
            slot_dst[c, w, :n] = c * npc + w * P + r_w
            dstrow[c, w, :n] = r_w
    # idx arrays (int16, wrapped 16-way, replicated across partition groups)
    SW = S // 16
    idx_w = np.zeros((NCORES, P, nw * SW), np.int16)
    for c in range(NCORES):
        for w in range(nw):
            a = slot_src[c, w].astype(np.int16).reshape(SW, 16)
            idx_w[c, :, w * SW:(w + 1) * SW] = np.tile(a.T, (8, 1))
    # dstloc layout, pair-duplicated for the DVE 2x packed compare:
    # [p, (w*K + k)*2 + {0,1}] = dstrow[w, k*128+p]
    dl = dstrow.reshape(NCORES, nw, K, P).transpose(0, 3, 1, 2).reshape(NCORES, P, nw * K)
    dl = np.repeat(dl, 2, axis=2).astype(BF)
    return K, nw, npc_pad, kreal, slot_src, slot_dst, idx_w, dl


def _expand_pairs(slot_src, slot_dst, asrc, adst, nw, K):
    """Per-slot alpha = asrc[src] + adst[dst] -> [P, nw*K*H] bf16 (0 for pads)."""
    H = asrc.shape[1]
    s = slot_src.reshape(-1)
    d = slot_dst.reshape(-1)
    valid = d >= 0
    vals = np.zeros((s.shape[0], H), np.float32)
    vals[valid] = asrc[s[valid]] + adst[d[valid]]
    # [nw, K, 128, H] -> [128, nw, K, H], pair-duplicated along H for the
    # DVE 2x packed multiply
    out = vals.reshape(nw, K, P, H).transpose(2, 0, 1, 3).reshape(P, nw * K * H)
    return np.repeat(out, 2, axis=1).astype(BF)


def _asd_blockdiag(a_src, a_dst):
    H, C = a_src.shape
    out = np.zeros((H * C, 2 * H), np.float32)
    for h in range(H):
        out[h * C:(h + 1) * C, h] = a_src[h]
        out[h * C:(h + 1) * C, H + h] = a_dst[h]
    return out


SUBCH = 6  # gather chunks (x128 idxs) per dma_gather call; 48 desc/engine


def _pair_bcast(ap, rep):
    """From [..., n, 2] pair AP, build [..., n, rep, 2] with the rep dim at
    stride 0 — keeps the innermost read step-1 so DVE picks the 2x mode."""
    lay = list(ap.ap)
    return bass.AP(ap.tensor, ap.offset, lay[:-1] + [[0, rep], lay[-1]])


_QN = [0]


def _emit_window_gather(nc, G, tab, idx_sb, w, kw, SW, regs, elem):
    """Gather one window's kw*128 rows as ceil(kw/SUBCH) packed dma_gather
    calls (all slots valid; pads gather row 0), round-robin over the 4
    SWDGE queues (each runs on its own Q7 core pair)."""
    for s0 in range(0, kw, SUBCH):
        kk = min(SUBCH, kw - s0)
        lo = s0 * P
        nc.gpsimd.dma_gather(
            out_ap=G[:, s0 * elem:(s0 + kk) * elem].rearrange("p (k d) -> p k d", d=elem),
            in_ap=tab[:],
            idxs_ap=idx_sb[:, w * SW + lo // 16: w * SW + (lo + kk * P) // 16],
            num_idxs=kk * P,
            num_idxs_reg=regs[kk],
            elem_size=elem,
            single_packet=True,
            queue_num=_QN[0],
        )
        _QN[0] = (_QN[0] + 1) % 4


def _build_A(D1, H1, npc_pad):
    """h = x@W1 -> bf16 table [npc_pad, D1]; alphas = x@(W1@Asd) -> f32
    [P, nw*2*H1] (node t*128+p at column t*2*H1)."""
    nw = npc_pad // P
    KB = D1 // P
    nc = bass.Bass("TRN2", target_bir_lowering=False, debug=False, num_devices=NCORES)
    xT = nc.dram_tensor("xT", [D1, npc_pad], F32R, kind="ExternalInput")
    W1 = nc.dram_tensor("W1", [D1, D1], F32R, kind="ExternalInput")
    W1T = nc.dram_tensor("W1T", [D1, D1], F32R, kind="ExternalInput")
    Asd = nc.dram_tensor("Asd", [D1, 2 * H1], F32R, kind="ExternalInput")
    h_tab = nc.dram_tensor("h_tab", [npc_pad, D1], BF16, kind="ExternalOutput")
    aa1 = nc.dram_tensor("aa1", [P, nw * 2 * H1], F32, kind="ExternalOutput")
    with tile.TileContext(nc) as tc:
        with ExitStack() as ctx:
            const = ctx.enter_context(tc.tile_pool(name="const", bufs=1))
            work = ctx.enter_context(tc.tile_pool(name="work", bufs=3))
            ps = ctx.enter_context(tc.tile_pool(name="ps", bufs=2, space="PSUM"))
            ps2 = ctx.enter_context(tc.tile_pool(name="ps2", bufs=2, space="PSUM"))
            # per-node-tile layout: xsb[p, (t*KB + b)*P + j] = xT[b*P+p, t*P+j],
            # loaded tile-by-tile so the first matmuls start immediately
            xsb = const.tile([P, KB * npc_pad], F32R)
            for t_i in range(nw):
                nc.sync.dma_start(
                    out=xsb[:, t_i * KB * P:(t_i + 1) * KB * P].rearrange(
                        "p (b n) -> p b n", b=KB),
                    in_=xT[:, t_i * P:(t_i + 1) * P].rearrange(
                        "(b p) n -> p b n", p=P))
            w1_sb, w1t_sb, asd_sb = [], [], []
            for kb in range(KB):
                t = const.tile([P, D1], F32R, tag=f"w1_{kb}")
                nc.sync.dma_start(out=t[:], in_=W1[kb * P:(kb + 1) * P, :])
                w1_sb.append(t)
                t2 = const.tile([P, D1], F32R, tag=f"w1t_{kb}")
                nc.sync.dma_start(out=t2[:], in_=W1T[kb * P:(kb + 1) * P, :])
                w1t_sb.append(t2)
                t3 = const.tile([P, 2 * H1], F32R, tag=f"asd_{kb}")
                nc.sync.dma_start(out=t3[:], in_=Asd[kb * P:(kb + 1) * P, :])
                asd_sb.append(t3)
            wsd_sb = []
            for ib in range(KB):
                pw = ps2.tile([P, 2 * H1], F32, tag="pa")
                for cb in range(KB):
                    nc.tensor.matmul(out=pw[:], lhsT=w1t_sb[cb][:, ib * P:(ib + 1) * P],
                                     rhs=asd_sb[cb][:], start=cb == 0, stop=cb == KB - 1)
                t = const.tile([P, 2 * H1], F32R, tag=f"wsd_{ib}")
                nc.scalar.activation(out=t[:], in_=pw[:], func=AF.Copy)
                wsd_sb.append(t)
            aa_acc = const.tile([P, nw * 2 * H1], F32)
            for t_i in range(nw):
                ph = ps.tile([P, D1], F32, tag="ph")
                pa = ps2.tile([P, 2 * H1], F32, tag="pa")
                for kb in range(KB):
                    xt = xsb[:, (t_i * KB + kb) * P:(t_i * KB + kb + 1) * P]
                    nc.tensor.matmul(out=ph[:], lhsT=xt, rhs=w1_sb[kb][:],
                                     start=kb == 0, stop=kb == KB - 1)
                    nc.tensor.matmul(out=pa[:], lhsT=xt, rhs=wsd_sb[kb][:],
                                     start=kb == 0, stop=kb == KB - 1)
                stage = work.tile([P, D1], BF16, tag="stage")
                nc.scalar.activation(out=stage[:], in_=ph[:], func=AF.Copy)
                nc.sync.dma_start(out=h_tab[t_i * P:(t_i + 1) * P, :], in_=stage[:])
                nc.vector.tensor_copy(
                    out=aa_acc[:, t_i * 2 * H1:(t_i + 1) * 2 * H1], in_=pa[:])
            nc.sync.dma_start(out=aa1[:, :], in_=aa_acc[:])
    _split_excess_waits(nc)
    return nc


def _build_B(N, D1, H1, OUTC, npc_pad, K, kreal, with_b1):
    """Layer-1 edge phase + [W2|att2] transform producing the layer-2 table."""
    nw = npc_pad // P
    C1 = D1 // H1
    S = K * P
    SW = S // 16
    OB = OUTC // P
    KB = D1 // P
    nc = bass.Bass("TRN2", target_bir_lowering=False, debug=False, num_devices=NCORES,
                   num_swdge_queues=4)
    tab = nc.dram_tensor("tab", [N, D1], BF16, kind="ExternalInput")
    idxs = nc.dram_tensor("idxs", [P, nw * SW], I16, kind="ExternalInput")
    dlt = nc.dram_tensor("dlt", [P, nw * K * 2], BF16, kind="ExternalInput")
    aab = nc.dram_tensor("aab", [P, nw * K * H1 * 2], BF16, kind="ExternalInput")
    iotg = nc.dram_tensor("iotg", [P, K * P], BF16, kind="ExternalInput")
    idf = nc.dram_tensor("idf", [P, P], F32, kind="ExternalInput")
    W2 = nc.dram_tensor("W2", [D1, OUTC], F32R, kind="ExternalInput")
    W2T = nc.dram_tensor("W2T", [OUTC, D1], F32R, kind="ExternalInput")
    A2 = nc.dram_tensor("A2", [OUTC, 2], F32R, kind="ExternalInput")
    if with_b1:
        B1 = nc.dram_tensor("B1", [P, D1], F32, kind="ExternalInput")
    tab2 = nc.dram_tensor("tab2", [npc_pad, OUTC], BF16, kind="ExternalOutput")
    aa2 = nc.dram_tensor("aa2", [P, nw * 2], F32, kind="ExternalOutput")
    with tile.TileContext(nc) as tc:
        with ExitStack() as ctx:
            const = ctx.enter_context(tc.tile_pool(name="const", bufs=1))
            gp = ctx.enter_context(tc.tile_pool(name="gp", bufs=3))
            mp = ctx.enter_context(tc.tile_pool(name="mp", bufs=2))
            cp = ctx.enter_context(tc.tile_pool(name="cp", bufs=2))
            sp = ctx.enter_context(tc.tile_pool(name="sp", bufs=2))
            fp = ctx.enter_context(tc.tile_pool(name="fp", bufs=2))
            ps_po = ctx.enter_context(tc.tile_pool(name="ps_po", bufs=2, space="PSUM"))
            ps_pd = ctx.enter_context(tc.tile_pool(name="ps_pd", bufs=2, space="PSUM"))
            ps_h2 = ctx.enter_context(tc.tile_pool(name="ps_h2", bufs=2, space="PSUM"))
            ps_ct = ctx.enter_context(tc.tile_pool(name="ps_ct", bufs=2, space="PSUM"))

            nc.gpsimd.load_library(_mlp_lib)
            idx_sb = const.tile([P, nw * SW], I16)
            nc.sync.dma_start(out=idx_sb[:], in_=idxs[:, :])
            dl_sb = const.tile([P, nw * K * 2], BF16)
            nc.sync.dma_start(out=dl_sb[:], in_=dlt[:, :])
            aab_sb = const.tile([P, nw * K * H1 * 2], BF16)
            nc.sync.dma_start(out=aab_sb[:], in_=aab[:, :])
            if with_b1:
                bb = const.tile([P, D1], F32)
                nc.sync.dma_start(out=bb[:], in_=B1[:, :])
            iotag = const.tile([P, K * P], BF16)
            nc.sync.dma_start(out=iotag[:], in_=iotg[:, :])
            identF = const.tile([P, P], F32)
            nc.sync.dma_start(out=identF[:], in_=idf[:, :])
            w2t_sb, a2_sb = [], []
            for ob in range(OB):
                t = const.tile([P, D1], F32R, tag=f"w2t_{ob}")
                nc.sync.dma_start(out=t[:], in_=W2T[ob * P:(ob + 1) * P, :])
                w2t_sb.append(t)
                t2 = const.tile([P, 2], F32R, tag=f"a2_{ob}")
                nc.sync.dma_start(out=t2[:], in_=A2[ob * P:(ob + 1) * P, :])
                a2_sb.append(t2)
            w2ext_sb = []
            for ib in range(KB):
                pv = ps_h2.tile([P, OUTC + 2], F32, tag="ph2")
                for ob in range(OB):
                    nc.tensor.matmul(out=pv[:, :2], lhsT=w2t_sb[ob][:, ib * P:(ib + 1) * P],
                                     rhs=a2_sb[ob][:], start=ob == 0, stop=ob == OB - 1)
                t = const.tile([P, OUTC + 2], F32R, tag=f"w2e_{ib}")
                nc.sync.dma_start(out=t[:, :OUTC], in_=W2[ib * P:(ib + 1) * P, :])
                nc.scalar.activation(out=t[:, OUTC:OUTC + 2], in_=pv[:, :2], func=AF.Copy)
                w2ext_sb.append(t)

            kks = set()
            for kw in kreal:
                kks.update(min(SUBCH, kw - s0) for s0 in range(0, kw, SUBCH))
            regs = {kk: nc.gpsimd.to_reg(kk * P) for kk in kks}
            aa2_acc = const.tile([P, nw * 2], F32)

            def flush_b(w, po, pd):
                den = fp.tile([P, H1 * 2], F32, tag="den", name="den")
                nc.vector.tensor_scalar(out=den[:], in0=pd[:], scalar1=1e-16,
                                        scalar2=None, op0=OP.add)
                den_r = fp.tile([P, H1 * 2], F32, tag="den_r", name="den_r")
                nc.vector.reciprocal(out=den_r[:], in_=den[:])
                dv = den_r[:]
                den_r8 = bass.AP(dv.tensor, dv.offset,
                                 [list(dv.ap)[0], [2, H1], [0, C1]])
                o1 = fp.tile([P, D1], F32, tag="o1", name="o1")
                nc.vector.tensor_tensor(
                    out=o1[:].rearrange("p (h c) -> p h c", h=H1),
                    in0=po[:].rearrange("p (h c) -> p h c", h=H1),
                    in1=den_r8, op=OP.mult)
                if with_b1:
                    nc.vector.tensor_tensor(out=o1[:], in0=o1[:], in1=bb[:], op=OP.add)
                ee = fp.tile([P, D1], F32, tag="ee", name="ee")
                nc.scalar.activation(out=ee[:], in_=o1[:], func=AF.Exp)
                em = fp.tile([P, D1], F32, tag="em", name="em")
                nc.vector.tensor_scalar(out=em[:], in0=ee[:], scalar1=-1.0,
                                        scalar2=None, op0=OP.add)
                nc.vector.tensor_scalar(out=em[:], in0=em[:], scalar1=0.0,
                                        scalar2=None, op0=OP.min)
                h2 = fp.tile([P, D1], F32, tag="h2", name="h2")
                nc.vector.tensor_tensor(out=h2[:], in0=o1[:], in1=em[:], op=OP.max)
                ph2 = ps_h2.tile([P, OUTC + 2], F32, tag="ph2", name="ph2")
                for cb in range(KB):
                    pt = ps_ct.tile([P, P], F32, tag="ct", name="pt")
                    nc.tensor.transpose(out=pt[:], in_=h2[:, cb * P:(cb + 1) * P],
                                        identity=identF[:])
                    h2t = cp.tile([P, P], F32R, tag="h2t", name="h2t")
                    nc.scalar.activation(out=h2t[:], in_=pt[:], func=AF.Copy)
                    nc.tensor.matmul(out=ph2[:], lhsT=h2t[:], rhs=w2ext_sb[cb][:],
                                     start=cb == 0, stop=cb == KB - 1)
                stage = fp.tile([P, OUTC], BF16, tag="stage", name="stage")
                nc.scalar.activation(out=stage[:], in_=ph2[:, :OUTC], func=AF.Copy)
                nc.sync.dma_start(out=tab2[w * P:(w + 1) * P, :], in_=stage[:])
                nc.vector.tensor_copy(out=aa2_acc[:, w * 2:(w + 1) * 2],
                                      in_=ph2[:, OUTC:OUTC + 2])

            pend = None
            for w in range(nw):
                kw = kreal[w]
                G = gp.tile([P, K * D1], BF16, tag="G")
                _emit_window_gather(nc, G, tab, idx_sb, w, kw, SW, regs, D1)
                lr = sp.tile([P, K * H1 * 2], BF16, tag="lr")
                nc.scalar.activation(out=lr[:, :kw * H1 * 2],
                                     in_=aab_sb[:, w * K * H1 * 2: (w * K + kw) * H1 * 2],
                                     func=AF.Prelu, alpha=NEG_SLOPE)
                ex = sp.tile([P, K * H1 * 2], BF16, tag="ex")
                nc.scalar.activation(out=ex[:, :kw * H1 * 2], in_=lr[:, :kw * H1 * 2],
                                     func=AF.Exp)
                CMP = cp.tile([P, K * P], BF16, tag="CMP")
                nc.vector.tensor_tensor(
                    out=CMP[:, :kw * P].rearrange("p (k q2 two) -> p k q2 two",
                                                  k=kw, q2=P // 2),
                    in0=iotag[:, :kw * P].rearrange("p (k q2 two) -> p k q2 two",
                                                    k=kw, q2=P // 2),
                    in1=_pair_bcast(
                        dl_sb[:, w * K * 2: (w * K + kw) * 2]
                        .rearrange("p (k two) -> p k two", k=kw), P // 2),
                    op=OP.is_equal)
                M = mp.tile([P, K * D1], BF16, tag="M")
                nc.vector.tensor_tensor(
                    out=M[:, :kw * D1].rearrange("p (k h c2 two) -> p k h c2 two",
                                                 k=kw, h=H1, c2=C1 // 2),
                    in0=G[:, :kw * D1].rearrange("p (k h c2 two) -> p k h c2 two",
                                                 k=kw, h=H1, c2=C1 // 2),
                    in1=_pair_bcast(
                        ex[:, :kw * H1 * 2]
                        .rearrange("p (k h two) -> p k h two", k=kw, h=H1), C1 // 2),
                    op=OP.mult)
                po = ps_po.tile([P, D1], F32, tag="po")
                pd = ps_pd.tile([P, H1 * 2], F32, tag="pd")
                for k in range(kw):
                    nc.tensor.matmul(out=po[:], lhsT=CMP[:, k * P:(k + 1) * P],
                                     rhs=M[:, k * D1:(k + 1) * D1],
                                     start=k == 0, stop=k == kw - 1)
                    nc.tensor.matmul(out=pd[:], lhsT=CMP[:, k * P:(k + 1) * P],
                                     rhs=ex[:, k * H1 * 2:(k + 1) * H1 * 2],
                                     start=k == 0, stop=k == kw - 1)
                if pend is not None:
                    flush_b(*pend)
                pend = (w, po, pd)
            flush_b(*pend)
            nc.sync.dma_start(out=aa2[:, :], in_=aa2_acc[:])
    _split_excess_waits(nc)
    _lower_ext(nc)
    return nc


def _build_C(N, OUTC, npc_pad, K, kreal, with_b2):
    """Layer-2 edge phase: exp(alpha2) folded into the selector; denominator
    via a ones column; flush = divide, (+b2), log_softmax."""
    nw = npc_pad // P
    S = K * P
    SW = S // 16
    nc = bass.Bass("TRN2", target_bir_lowering=False, debug=False, num_devices=NCORES,
                   num_swdge_queues=4)
    tab2 = nc.dram_tensor("tab2", [N, OUTC], BF16, kind="ExternalInput")
    idxs = nc.dram_tensor("idxs", [P, nw * SW], I16, kind="ExternalInput")
    dlt = nc.dram_tensor("dlt", [P, nw * K * 2], BF16, kind="ExternalInput")
    aab2 = nc.dram_tensor("aab2", [P, nw * K * 2], BF16, kind="ExternalInput")
    iotg = nc.dram_tensor("iotg", [P, K * P], BF16, kind="ExternalInput")
    if with_b2:
        B2 = nc.dram_tensor("B2", [P, OUTC], F32, kind="ExternalInput")
    out_t = nc.dram_tensor("out", [npc_pad, OUTC], F32, kind="ExternalOutput")
    with tile.TileContext(nc) as tc:
        with ExitStack() as ctx:
            const = ctx.enter_context(tc.tile_pool(name="const", bufs=1))
            gp = ctx.enter_context(tc.tile_pool(name="gp", bufs=3))
            cp = ctx.enter_context(tc.tile_pool(name="cp", bufs=2))
            sp = ctx.enter_context(tc.tile_pool(name="sp", bufs=2))
            fp = ctx.enter_context(tc.tile_pool(name="fp", bufs=2))
            ps_po = ctx.enter_context(tc.tile_pool(name="ps_po", bufs=2, space="PSUM"))
            ps_pd = ctx.enter_context(tc.tile_pool(name="ps_pd", bufs=2, space="PSUM"))

            nc.gpsimd.load_library(_mlp_lib)
            idx_sb = const.tile([P, nw * SW], I16)
            nc.sync.dma_start(out=idx_sb[:], in_=idxs[:, :])
            dl_sb = const.tile([P, nw * K * 2], BF16)
            nc.sync.dma_start(out=dl_sb[:], in_=dlt[:, :])
            aab_sb = const.tile([P, nw * K * 2], BF16)
            nc.sync.dma_start(out=aab_sb[:], in_=aab2[:, :])
            if with_b2:
                bb = const.tile([P, OUTC], F32)
                nc.sync.dma_start(out=bb[:], in_=B2[:, :])
            iotag = const.tile([P, K * P], BF16)
            nc.sync.dma_start(out=iotag[:], in_=iotg[:, :])
            ones = const.tile([P, 1], BF16)
            nc.vector.memset(ones[:], 1.0)

            kks = set()
            for kw in kreal:
                kks.update(min(SUBCH, kw - s0) for s0 in range(0, kw, SUBCH))
            regs = {kk: nc.gpsimd.to_reg(kk * P) for kk in kks}

            def flush_c1(w, po, pd):
                den = fp.tile([P, 1], F32, tag="den", name="den")
                nc.vector.tensor_scalar(out=den[:], in0=pd[:], scalar1=1e-16,
                                        scalar2=None, op0=OP.add)
                den_r = fp.tile([P, 1], F32, tag="den_r", name="den_r")
                nc.vector.reciprocal(out=den_r[:], in_=den[:])
                z = fp.tile([P, OUTC], F32, tag="z", name="z")
                nc.vector.tensor_scalar(out=z[:], in0=po[:], scalar1=den_r[:, :1],
                                        scalar2=None, op0=OP.mult)
                if with_b2:
                    nc.vector.tensor_tensor(out=z[:], in0=z[:], in1=bb[:], op=OP.add)
                ee = fp.tile([P, OUTC], F32, tag="ee", name="ee")
                se = fp.tile([P, 1], F32, tag="se", name="se")
                nc.scalar.activation(out=ee[:], in_=z[:], func=AF.Exp, accum_out=se[:])
                lse = fp.tile([P, 1], F32, tag="lse", name="lse")
                nc.scalar.activation(out=lse[:], in_=se[:], func=AF.Ln)
                return (w, z, lse)

            def flush_c2(w, z, lse):
                zf = fp.tile([P, OUTC], F32, tag="zf", name="zf")
                nc.vector.tensor_scalar(out=zf[:], in0=z[:], scalar1=lse[:, :1],
                                        scalar2=None, op0=OP.subtract)
                nc.sync.dma_start(out=out_t[w * P:(w + 1) * P, :], in_=zf[:])

            pend = None
            pend2 = None
            for w in range(nw):
                kw = kreal[w]
                G = gp.tile([P, K * OUTC], BF16, tag="G")
                _emit_window_gather(nc, G, tab2, idx_sb, w, kw, SW, regs, OUTC)
                lr = sp.tile([P, K * 2], BF16, tag="lr")
                nc.scalar.activation(out=lr[:, :kw * 2],
                                     in_=aab_sb[:, w * K * 2: (w * K + kw) * 2],
                                     func=AF.Prelu, alpha=NEG_SLOPE)
                ex = sp.tile([P, K * 2], BF16, tag="ex")
                nc.scalar.activation(out=ex[:, :kw * 2], in_=lr[:, :kw * 2], func=AF.Exp)
                CMP = cp.tile([P, K * P], BF16, tag="CMP")
                nc.vector.tensor_tensor(
                    out=CMP[:, :kw * P].rearrange("p (k q2 two) -> p k q2 two",
                                                  k=kw, q2=P // 2),
                    in0=iotag[:, :kw * P].rearrange("p (k q2 two) -> p k q2 two",
                                                    k=kw, q2=P // 2),
                    in1=_pair_bcast(
                        dl_sb[:, w * K * 2: (w * K + kw) * 2]
                        .rearrange("p (k two) -> p k two", k=kw), P // 2),
                    op=OP.is_equal)
                CMX = cp.tile([P, K * P], BF16, tag="CMX")
                nc.vector.tensor_tensor(
                    out=CMX[:, :kw * P].rearrange("p (k q2 two) -> p k q2 two",
                                                  k=kw, q2=P // 2),
                    in0=CMP[:, :kw * P].rearrange("p (k q2 two) -> p k q2 two",
                                                  k=kw, q2=P // 2),
                    in1=_pair_bcast(
                        ex[:, :kw * 2].rearrange("p (k two) -> p k two", k=kw), P // 2),
                    op=OP.mult)
                po = ps_po.tile([P, OUTC], F32, tag="po")
                pd = ps_pd.tile([P, 1], F32, tag="pd")
                for k in range(kw):
                    nc.tensor.matmul(out=po[:], lhsT=CMX[:, k * P:(k + 1) * P],
                                     rhs=G[:, k * OUTC:(k + 1) * OUTC],
                                     start=k == 0, stop=k == kw - 1)
                    nc.tensor.matmul(out=pd[:], lhsT=CMX[:, k * P:(k + 1) * P],
                                     rhs=ones[:], start=k == 0, stop=k == kw - 1)
                if pend2 is not None:
                    flush_c2(*pend2)
                    pend2 = None
                if pend is not None:
                    pend2 = flush_c1(*pend)
                pend = (w, po, pd)
            if pend2 is not None:
                flush_c2(*pend2)
            flush_c2(*flush_c1(*pend))
    _split_excess_waits(nc)
    _lower_ext(nc)
    return nc


def kernel(x, edge_index, W1, att_src1, att_dst1, b1, W2, att_src2, att_dst2, b2):
    x = np.asarray(x, np.float32)
    edge_index = np.asarray(edge_index)
    W1 = np.asarray(W1, np.float32)
    W2 = np.asarray(W2, np.float32)
    att_src1 = np.asarray(att_src1, np.float32)
    att_dst1 = np.asarray(att_dst1, np.float32)
    att_src2 = np.asarray(att_src2, np.float32)
    att_dst2 = np.asarray(att_dst2, np.float32)
    b1 = np.asarray(b1, np.float32)
    b2 = np.asarray(b2, np.float32)
    N, D1 = x.shape
    H1 = att_src1.shape[0]
    OUTC = W2.shape[1]
    npc = N // NCORES
    core_ids = list(range(NCORES))
    with_b1 = bool(np.any(b1))
    with_b2 = bool(np.any(b2))

    K, nw, npc_pad, kreal, slot_src, slot_dst, idx_w, dl = _preprocess(edge_index, N, npc)
    asd = _asd_blockdiag(att_src1, att_dst1)
    a2 = np.stack([att_src2[0], att_dst2[0]], axis=1)

    # ---- launch A
    nc_a = _build_A(D1, H1, npc_pad)
    in_maps = []
    for c in range(NCORES):
        xo = np.zeros((npc_pad, D1), np.float32)
        xo[:npc] = x[c * npc:(c + 1) * npc]
        in_maps.append({"xT": np.ascontiguousarray(xo.T), "W1": W1,
                        "W1T": np.ascontiguousarray(W1.T), "Asd": asd})
    res_a = run_bass_kernel_spmd(nc_a, in_maps, core_ids)
    h_full = np.concatenate(
        [res_a.results[c]["h_tab"][:npc] for c in range(NCORES)], axis=0)
    # per-node alpha terms [N, H1] each
    asrc1 = np.zeros((N, H1), np.float32)
    adst1 = np.zeros((N, H1), np.float32)
    for c in range(NCORES):
        aa = res_a.results[c]["aa1"].reshape(P, nw, 2 * H1).transpose(1, 0, 2).reshape(npc_pad, 2 * H1)
        asrc1[c * npc:(c + 1) * npc] = aa[:npc, :H1]
        adst1[c * npc:(c + 1) * npc] = aa[:npc, H1:]

    iotg = np.tile(np.arange(P, dtype=np.float32), (P, K)).astype(BF)
    idf = np.eye(P, dtype=np.float32)

    # ---- launch B
    nc_b = _build_B(N, D1, H1, OUTC, npc_pad, K, kreal, with_b1)
    in_maps = []
    for c in range(NCORES):
        m = {"tab": h_full, "idxs": idx_w[c], "dlt": dl[c],
             "aab": _expand_pairs(slot_src[c], slot_dst[c], asrc1, adst1, nw, K),
             "iotg": iotg, "idf": idf,
             "W2": W2, "W2T": np.ascontiguousarray(W2.T), "A2": a2}
        if with_b1:
            m["B1"] = np.tile(b1.reshape(1, D1), (P, 1))
        in_maps.append(m)
    res_b = run_bass_kernel_spmd(nc_b, in_maps, core_ids)
    t2_full = np.concatenate(
        [res_b.results[c]["tab2"][:npc] for c in range(NCORES)], axis=0)
    asrc2 = np.zeros((N, 1), np.float32)
    adst2 = np.zeros((N, 1), np.float32)
    for c in range(NCORES):
        aa = res_b.results[c]["aa2"].reshape(P, nw, 2).transpose(1, 0, 2).reshape(npc_pad, 2)
        asrc2[c * npc:(c + 1) * npc] = aa[:npc, :1]
        adst2[c * npc:(c + 1) * npc] = aa[:npc, 1:]

    # ---- launch C
    nc_c = _build_C(N, OUTC, npc_pad, K, kreal, with_b2)
    in_maps = []
    for c in range(NCORES):
        m = {"tab2": t2_full, "idxs": idx_w[c], "dlt": dl[c],
             "aab2": _expand_pairs(slot_src[c], slot_dst[c], asrc2, adst2, nw, K),
             "iotg": iotg}
        if with_b2:
            m["B2"] = np.tile(b2.reshape(1, OUTC), (P, 1))
        in_maps.append(m)
    res_c = run_bass_kernel_spmd(nc_c, in_maps, core_ids)
    out = np.concatenate(
        [res_c.results[c]["out"][:npc] for c in range(NCORES)], axis=0)
    return out.astype(np.float32)
